# revision 12
# baseline (speedup 1.0000x reference)
"""Trainium2 Bass kernel for the patch-CNN problem (v3: compensated-fp8).

Math: x [2048,2,25,25] --bicubic-up--> [2048,2,50,50] --5x5-grid of 10x10
patches--> per-patch 3-layer CNN (two weight branches phi/p, 25 patch-specific
weight sets each) --> reassemble [2048,2,50,50].

Every stage (upsample, conv1, conv2, conv3) is a banded matrix acting on
activation vectors laid out [(spatial, channel), batch] in SBUF; batch is the
matmul moving dimension. Band matrices are built on the host from the input
weights and cut into 128x128 blocks; only structurally-nonzero blocks execute.

v3 over v2: up/c1/c2 matmuls run in float8e5 DoubleRow perf mode with full
error compensation. Each weight block W ~ W_hi + W_lo (both e5m2), each
activation X ~ X_hi + X_lo (e5m2); block pairs (a,b) execute as three
DoubleRow matmuls (107ns each vs 213ns per bf16 block):
    (Whi_a@Xhi_a + Whi_b@Xhi_b), (Wlo_a@Xhi_a + Wlo_b@Xhi_b),
    (Whi_a@Xlo_a + Whi_b@Xlo_b)
and odd blocks as two: ([hi|hi], (Xhi,Xlo)) + ([lo|0], (Xhi,Xlo)).
c3 stays bf16 (its h2 quantization passes would cost more than the PE gain).
All k-chunks are zero-padded to 128 rows so any two blocks can pair.
End-to-end rel err ~9.7e-3 (gate 2e-2); PE time ~700us/core vs 852 bf16.
"""

import numpy as np
from contextlib import ExitStack

import ml_dtypes

import concourse.bass as bass
import concourse.mybir as mybir
import concourse.tile as tile
from concourse import bacc
from concourse.bass_utils import run_bass_kernel_spmd

# ---------------------------------------------------------------- constants
B = 2048
C_IN = 2
W1C = 16
W2C = 32
KK = 5
NP = 25
L = 10
OUT = 50
N_CORES = 8
BQ = 512               # images per slot (batch quarter; also the moving tile)

H_WIN = 9              # input rows needed per patch row (bicubic support)
XS_ROWS = H_WIN * 25 * C_IN   # 450 real rows
XS_PAD = 512                  # padded to 4 k-tiles
XUP_ROWS = 200                # (y'',x'',ci) per patch
H1_ROWS = 100 * W1C           # 1600
H2_ROWS = 100 * W2C           # 3200
OUT_ROWS = 100
NCH_XS = 4
NCH_XUP = 2
NCH_H1 = 13
NCH_H2 = 25

f32 = mybir.dt.float32
bf16 = mybir.dt.bfloat16
e5 = mybir.dt.float8e5
BF16 = ml_dtypes.bfloat16
E5 = ml_dtypes.float8_e5m2
DR = mybir.MatmulPerfMode.DoubleRow


def _h0(i):
    return min(max(5 * i - 2, 0), 25 - H_WIN)


def bicubic_matrix(out_size, in_size):
    """Row-stochastic resize matrix identical to jax.image.resize bicubic."""
    scale = out_size / in_size
    u = (np.arange(out_size) + 0.5) / scale - 0.5
    s = np.abs(u[:, None] - np.arange(in_size)[None, :])
    A = -0.5
    w = np.where(
        s <= 1,
        (A + 2) * s**3 - (A + 3) * s**2 + 1,
        np.where(s < 2, A * s**3 - 5 * A * s**2 + 8 * A * s - 4 * A, 0.0),
    )
    w = w / w.sum(axis=1, keepdims=True)
    return w.astype(np.float64)


_R = bicubic_matrix(OUT, 25)           # [50, 25]
_R_NZ = np.abs(_R) > 1e-300


# pixel-order permutations: h1 rows use 4x2-brick order, h2 rows 2x2-brick
# order; minimizes touched 128x128 blocks in the c2 band matrix.
PERM_ID = np.arange(100)
PERM_H1 = np.array([(4 * by + dy) * 10 + 2 * bx + dx
                    for by in range(3) for bx in range(5)
                    for dy in range(4) for dx in range(2)
                    if 4 * by + dy < 10])
PERM_H2 = np.array([(2 * by + dy) * 10 + 2 * bx + dx
                    for by in range(5) for bx in range(5)
                    for dy in range(2) for dx in range(2)])

# ------------------------------------------------------- structural schedule
def _grids(krows, mrows, kc, mc, kh, mw):
    kl = kc * 128 + np.arange(kh)
    ml = mc * 128 + np.arange(mw)
    return kl[:, None], ml[None, :]


def _conv_struct(krows, mrows, kdiv, mdiv, pk, pm):
    k = np.arange(krows)
    m = np.arange(mrows)
    sp, s = pk[k // kdiv], pm[m // mdiv]
    yp, xp = sp // 10, sp % 10
    y, x = s // 10, s % 10
    return (np.abs(yp[:, None] - y[None, :]) <= 2) & (
        np.abs(xp[:, None] - x[None, :]) <= 2
    )


def _up_struct():
    nz = np.zeros((XS_PAD, XUP_ROWS), dtype=bool)
    k = np.arange(XS_ROWS)
    hl, wv, ci = k // 50, (k % 50) // 2, k % 2
    m = np.arange(XUP_ROWS)
    spp, cip = m // 2, m % 2
    ypp, xpp = spp // 10, spp % 10
    for i in range(5):
        for j in range(5):
            lv = _R_NZ[10 * i + ypp, :][:, _h0(i) + hl]
            rv = _R_NZ[10 * j + xpp, :][:, wv]
            nzij = (lv & rv).T & (ci[:, None] == cip[None, :])
            nz[:XS_ROWS] |= nzij
    return nz


def _blocks_of(nzmask, krows, mrows):
    nkc = (krows + 127) // 128
    nmc = (mrows + 127) // 128
    out = []
    for mc in range(nmc):
        mw = min(128, mrows - mc * 128)
        kcs = []
        for kc in range(nkc):
            kh = min(128, krows - kc * 128)
            if nzmask[kc * 128 : kc * 128 + kh, mc * 128 : mc * 128 + mw].any():
                kcs.append((kc, kh))
        out.append((mw, kcs))
    return out


_SCHED = {
    "up": _blocks_of(_up_struct(), XS_PAD, XUP_ROWS),
    "c1": _blocks_of(_conv_struct(XUP_ROWS, H1_ROWS, 2, W1C, PERM_ID, PERM_H1),
                     XUP_ROWS, H1_ROWS),
    "c2": _blocks_of(_conv_struct(H1_ROWS, H2_ROWS, W1C, W2C, PERM_H1, PERM_H2),
                     H1_ROWS, H2_ROWS),
    "c3": _blocks_of(_conv_struct(H2_ROWS, OUT_ROWS, W2C, 1, PERM_H2, PERM_ID),
                     H2_ROWS, OUT_ROWS),
}


def _up_kcs_for_patch(p):
    """Nonzero k-chunks of the upsample band for patch p, per m-chunk."""
    i, j = p // 5, p % 5
    k = np.arange(XS_ROWS)
    hl, wv, ci = k // 50, (k % 50) // 2, k % 2
    m = np.arange(XUP_ROWS)
    spp, cip = m // 2, m % 2
    ypp, xpp = spp // 10, spp % 10
    lv = _R_NZ[10 * i + ypp, :][:, _h0(i) + hl]
    rv = _R_NZ[10 * j + xpp, :][:, wv]
    nz = np.zeros((XS_PAD, XUP_ROWS), bool)
    nz[:XS_ROWS] = (lv & rv).T & (ci[:, None] == cip[None, :])
    out = []
    for mc, (mw, kcs) in enumerate(_SCHED["up"]):
        keep = []
        for kc, kh in kcs:
            if nz[kc * 128 : kc * 128 + kh, mc * 128 : mc * 128 + mw].any():
                keep.append(kc)
        out.append(keep)
    return out


# ----------------------------------------------- fp8 pair schedule + layout
# Each m-chunk of {up, c1, c2} becomes: pairs [(kcA, kcB)...] + optional odd
# kc. Weight columns per pair/odd: 512 e5 cols [hi_a|hi_b|lo_a|lo_b] (odd:
# [hi|hi|lo|0]). c3 keeps bf16 blocks of 128 cols each.
def _pairs_of(kcs):
    kc_list = [kc for kc, _kh in kcs]
    pairs = [(kc_list[2 * i], kc_list[2 * i + 1])
             for i in range(len(kc_list) // 2)]
    odd = kc_list[-1] if len(kc_list) % 2 else None
    return pairs, odd


# up: fixed per-patch budget of 2 groups (pairs+odd <= 2) per m-chunk.
UP_COLS_PER_MC = 2 * 512
_UP5 = []     # [patch][mc] -> (pairs, odd, base_col)
for _p in range(NP):
    per_mc = []
    for _mc, keeps in enumerate(_up_kcs_for_patch(_p)):
        pairs = [(keeps[2 * i], keeps[2 * i + 1])
                 for i in range(len(keeps) // 2)]
        odd = keeps[-1] if len(keeps) % 2 else None
        per_mc.append((pairs, odd, _mc * UP_COLS_PER_MC))
    _UP5.append(per_mc)
_UP_END = len(_SCHED["up"]) * UP_COLS_PER_MC

_SCHED5 = {}  # lay -> [ (pairs, odd, base_col) per mc ]  (cols within layer)
_col = 0
for _lay in ("c1", "c2"):
    groups = []
    for _mc, (_mw, _kcs) in enumerate(_SCHED[_lay]):
        pairs, odd = _pairs_of(_kcs)
        groups.append((pairs, odd, _col))
        _col += (len(pairs) + (1 if odd is not None else 0)) * 512
    _SCHED5[_lay] = groups
_C1_OFF = _UP_END
_C2_OFF = _UP_END + _SCHED5["c2"][0][2]
Q5COLS = _UP_END + _col

B16COLS = len(_SCHED["c3"][0][1]) * 128    # c3: 25 blocks x 128 cols

_N_MC = {lay: len(_SCHED[lay]) for lay in _SCHED}
_BIAS_COLS = _N_MC["c1"] + _N_MC["c2"] + 1
_BC1, _BC2, _BC3 = 0, _N_MC["c1"], _N_MC["c1"] + _N_MC["c2"]

# weight-DMA sub-transfers of the e5 array: [up+c1 | c2 in thirds]
# (all boundaries multiples of 512 cols)
_C2_LEN = Q5COLS - _C2_OFF
_T3 = (_C2_LEN // 3 // 512) * 512
_WSPLITS5 = [(0, _C2_OFF),
             (_C2_OFF, _C2_OFF + _T3),
             (_C2_OFF + _T3, _C2_OFF + 2 * _T3),
             (_C2_OFF + 2 * _T3, Q5COLS)]
Q5GRP = Q5COLS // 128          # weight tile laid out [128, Q5GRP, 128]

N_UNITS = 2 * NP


# ------------------------------------------------- host weight-block builder
def _conv_gather(krows, mrows, kdiv, mdiv, nci, kc, mc, kh, mw, pk, pm):
    kl, ml = _grids(krows, mrows, kc, mc, kh, mw)
    sp, ci = pk[kl // kdiv], kl % kdiv
    s, co = pm[ml // mdiv], ml % mdiv
    yp, xp = sp // 10, sp % 10
    y, x = s // 10, s % 10
    ky = yp - y + 2
    kx = xp - x + 2
    valid = (ky >= 0) & (ky < 5) & (kx >= 0) & (kx < 5)
    ky = np.clip(ky, 0, 4)
    kx = np.clip(kx, 0, 4)
    widx = ((co * nci + ci) * 5 + ky) * 5 + kx
    return widx, valid


_CONV_GATHER_CACHE = {}


def _conv_gather_cached(lay, kc, mc, kh, mw):
    key = (lay, kc, mc)
    if key not in _CONV_GATHER_CACHE:
        if lay == "c1":
            g = _conv_gather(XUP_ROWS, H1_ROWS, 2, W1C, C_IN, kc, mc, kh, mw,
                             PERM_ID, PERM_H1)
        elif lay == "c2":
            g = _conv_gather(H1_ROWS, H2_ROWS, W1C, W2C, W1C, kc, mc, kh, mw,
                             PERM_H1, PERM_H2)
        else:
            g = _conv_gather(H2_ROWS, OUT_ROWS, W2C, 1, W2C, kc, mc, kh, mw,
                             PERM_H2, PERM_ID)
        _CONV_GATHER_CACHE[key] = g
    return _CONV_GATHER_CACHE[key]


def _build_f32_blocks(w1f, w2f, w3f, ij):
    """Float32 band blocks in dense per-(lay,mc,kc) dict form.

    Returns blocks[(lay, mc, kc)] = [nu, 128, 128] f32 (zero padded).
    """
    nu = w1f.shape[0]
    blocks = {}

    # upsample blocks
    k = np.arange(XS_PAD)
    hl, wv, ci = k // 50, (k % 50) // 2, k % 2
    hl = np.where(k < XS_ROWS, hl, 0)
    m = np.arange(XUP_ROWS)
    spp, cip = m // 2, m % 2
    ypp, xpp = spp // 10, spp % 10
    iu = ij[:, 0]
    ju = ij[:, 1]
    h0u = np.minimum(np.maximum(5 * iu - 2, 0), 25 - H_WIN)
    left = _R[(10 * iu[:, None, None] + ypp[None, :, None]),
              (h0u[:, None, None] + hl[None, None, :])]
    left *= (np.arange(XS_PAD)[None, None, :] < XS_ROWS)
    right = _R[(10 * ju[:, None, None] + xpp[None, :, None]),
               wv[None, None, :]]
    same = (ci[None, :] == cip[:, None])[None, :, :]
    upmat = (left * right * same).transpose(0, 2, 1).astype(np.float32)
    for mc, (mw, kcs) in enumerate(_SCHED["up"]):
        for kc, kh in kcs:
            blk = np.zeros((nu, 128, 128), np.float32)
            blk[:, :kh, :mw] = upmat[:, kc * 128 : kc * 128 + kh,
                                     mc * 128 : mc * 128 + mw]
            blocks[("up", mc, kc)] = blk

    for lay, wf in (("c1", w1f), ("c2", w2f), ("c3", w3f)):
        for mc, (mw, kcs) in enumerate(_SCHED[lay]):
            for kc, kh in kcs:
                widx, valid = _conv_gather_cached(lay, kc, mc, kh, mw)
                blk = wf[:, widx.reshape(-1)].reshape(nu, kh, mw)
                blk = blk * valid[None, :, :]
                full = np.zeros((nu, 128, 128), np.float32)
                full[:, :kh, :mw] = blk
                blocks[(lay, mc, kc)] = full
    return blocks


def _build_wq(blocks, nu):
    """Pack fp8 hi/lo pair layout [nu,128,Q5COLS] + bf16 c3 [nu,128,B16COLS]."""
    w5 = np.zeros((nu, 128, Q5COLS), dtype=E5)
    w16 = np.zeros((nu, 128, B16COLS), dtype=BF16)

    def put_pair(base, a, b):
        hi_a = a.astype(E5)
        hi_b = b.astype(E5)
        lo_a = (a - hi_a.astype(np.float32)).astype(E5)
        lo_b = (b - hi_b.astype(np.float32)).astype(E5)
        w5[:, :, base : base + 128] = hi_a
        w5[:, :, base + 128 : base + 256] = hi_b
        w5[:, :, base + 256 : base + 384] = lo_a
        w5[:, :, base + 384 : base + 512] = lo_b

    def put_odd(base, a):
        hi = a.astype(E5)
        lo = (a - hi.astype(np.float32)).astype(E5)
        w5[:, :, base : base + 128] = hi
        w5[:, :, base + 128 : base + 256] = hi
        w5[:, :, base + 256 : base + 384] = lo
        # base+384..+512 stays zero

    # up: per-unit layouts differ by patch
    for u in range(nu):
        p = u % NP
        for mc, (pairs, odd, base) in enumerate(_UP5[p]):
            col = base
            for (a, b) in pairs:
                blk_a = blocks[("up", mc, a)][u]
                blk_b = blocks[("up", mc, b)][u]
                hi_a, hi_b = blk_a.astype(E5), blk_b.astype(E5)
                w5[u, :, col : col + 128] = hi_a
                w5[u, :, col + 128 : col + 256] = hi_b
                w5[u, :, col + 256 : col + 384] = (
                    blk_a - hi_a.astype(np.float32)).astype(E5)
                w5[u, :, col + 384 : col + 512] = (
                    blk_b - hi_b.astype(np.float32)).astype(E5)
                col += 512
            if odd is not None:
                blk = blocks[("up", mc, odd)][u]
                hi = blk.astype(E5)
                w5[u, :, col : col + 128] = hi
                w5[u, :, col + 128 : col + 256] = hi
                w5[u, :, col + 256 : col + 384] = (
                    blk - hi.astype(np.float32)).astype(E5)

    for lay in ("c1", "c2"):
        for mc, (pairs, odd, base) in enumerate(_SCHED5[lay]):
            col = _C1_OFF + base
            for (a, b) in pairs:
                put_pair(col, blocks[(lay, mc, a)], blocks[(lay, mc, b)])
                col += 512
            if odd is not None:
                put_odd(col, blocks[(lay, mc, odd)])

    # c3 bf16
    for idx, (kc, kh) in enumerate(_SCHED["c3"][0][1]):
        w16[:, :, idx * 128 : (idx + 1) * 128] = blocks[("c3", 0, kc)].astype(BF16)
    return w5.reshape(nu, 128, Q5GRP, 128), w16


# --------------------------------------------------------- device program
_NC_CACHE = None


def _emit_fp8_group(nc, ps, wt5, xq, nch, pairs, odd, base_col):
    """Emit the DoubleRow matmul group for one m-chunk (own psum group).

    xq: [128, 2*nch, BQ] e5 tile (hi at idx kc, lo at idx nch+kc).
    wt5: [128, Q5GRP, 128] e5 tile; cols at group g0=base_col//128:
    per pair 4 groups [hi_a|hi_b|lo_a|lo_b], odd 4 groups [hi|hi|lo|0].
    """
    n_instr = 3 * len(pairs) + (2 if odd is not None else 0)
    i = 0
    g = base_col // 128

    def mm(w_ap, x_ap):
        nonlocal i
        nc.tensor.matmul(ps[:, :], w_ap, x_ap,
                         start=(i == 0), stop=(i == n_instr - 1),
                         perf_mode=DR, skip_group_check=True)
        i += 1

    for (a, b) in pairs:
        hi_w = wt5[:, g : g + 2, :]
        lo_w = wt5[:, g + 2 : g + 4, :]
        xhi = xq[:, a : b + 1 : (b - a), :]
        xlo = xq[:, nch + a : nch + b + 1 : (b - a), :]
        mm(hi_w, xhi)       # Whi_a@Xhi_a + Whi_b@Xhi_b
        mm(lo_w, xhi)       # Wlo_a@Xhi_a + Wlo_b@Xhi_b
        mm(hi_w, xlo)       # Whi_a@Xlo_a + Whi_b@Xlo_b
        g += 4
    if odd is not None:
        hihi = wt5[:, g : g + 2, :]
        lo0 = wt5[:, g + 2 : g + 4, :]
        xodd = xq[:, odd : nch + odd + 1 : nch, :]
        mm(hihi, xodd)      # [Whi|Whi] x (Xhi,Xlo) = Whi@(Xhi+Xlo)
        mm(lo0, xodd)       # [Wlo|0]  x (Xhi,Xlo) = Wlo@Xhi


def _build_nc():
    nc = bacc.Bacc("TRN2", target_bir_lowering=False, debug=False,
                   num_devices=N_CORES)
    xs_d = nc.dram_tensor("xs", [5, 128, 2 * NCH_XS, BQ], e5,
                          kind="ExternalInput").ap()
    wb5_d = nc.dram_tensor("wb5", [NP, 128, Q5GRP, 128], e5,
                           kind="ExternalInput").ap()
    wb16_d = nc.dram_tensor("wb16", [NP, 128, B16COLS], bf16,
                            kind="ExternalInput").ap()
    bias_d = nc.dram_tensor("bias", [NP, 128, _BIAS_COLS], f32,
                            kind="ExternalInput").ap()
    y_d = nc.dram_tensor("y", [NP, OUT_ROWS, BQ], f32, kind="ExternalOutput").ap()

    AF = mybir.ActivationFunctionType
    ALU = mybir.AluOpType
    mw3, kcs3 = _SCHED["c3"][0]
    kh3 = dict(kcs3)
    n_c2 = _N_MC["c2"]

    with tile.TileContext(nc) as tc, ExitStack() as ctx:
        xs_pool = ctx.enter_context(tc.tile_pool(name="xs", bufs=2))
        xupq_pool = ctx.enter_context(tc.tile_pool(name="xupq", bufs=2))
        h1q_pool = ctx.enter_context(tc.tile_pool(name="h1q", bufs=2))
        ract_pool = ctx.enter_context(tc.tile_pool(name="ract", bufs=4))
        h2_pool = ctx.enter_context(tc.tile_pool(name="h2", bufs=6))
        w5_pool = ctx.enter_context(tc.tile_pool(name="w5", bufs=2))
        w16_pool = ctx.enter_context(tc.tile_pool(name="w16", bufs=2))
        bias_pool = ctx.enter_context(tc.tile_pool(name="bias", bufs=2))
        out_pool = ctx.enter_context(tc.tile_pool(name="out", bufs=2))
        ps_pool = ctx.enter_context(tc.tile_pool(name="ps", bufs=6, space="PSUM"))
        ps3_pool = ctx.enter_context(tc.tile_pool(name="ps3", bufs=2, space="PSUM"))

        # engine split: psum reads only on ACT/DVE (GPSIMD cannot touch PSUM);
        # hi = cheap DVE sbuf copy; lo = GPSIMD tensor_tensor (sbuf only)
        ract_engines = [nc.scalar, nc.vector]
        lo_engines = [nc.gpsimd]
        cnt = {"ract": 0, "lo": 0}

        def quant_passes(ps_ap, bias_ap, r_t, q_t, nch, mc, relu):
            """ract -> bf16; hi = e5(ract) -> q[:, mc]; lo -> q[:, nch+mc]."""
            eng = ract_engines[cnt["ract"] % len(ract_engines)]
            cnt["ract"] += 1
            if relu:
                if hasattr(eng, "activation"):
                    eng.activation(r_t[:, :], ps_ap, AF.Relu, bias=bias_ap)
                else:
                    eng.tensor_scalar(r_t[:, :], ps_ap, bias_ap, 0.0,
                                      ALU.add, ALU.max)
            else:
                if hasattr(eng, "activation"):
                    eng.activation(r_t[:, :], ps_ap, AF.Identity)
                else:
                    eng.tensor_copy(r_t[:, :], ps_ap)
            nc.vector.tensor_copy(q_t[:, mc, :], r_t[:, :])
            loe = lo_engines[cnt["lo"] % len(lo_engines)]
            cnt["lo"] += 1
            loe.tensor_tensor(q_t[:, nch + mc, :], r_t[:, :], q_t[:, mc, :],
                              ALU.subtract)

        for i in range(5):
            xs_t = xs_pool.tile([128, 2 * NCH_XS, BQ], e5, tag="xs")
            nc.sync.dma_start(out=xs_t[:], in_=xs_d[i])

            for j in range(5):
                u = 5 * i + j
                wt5 = w5_pool.tile([128, Q5GRP, 128], e5, tag="w5")
                for c0, c1 in _WSPLITS5:
                    nc.gpsimd.dma_start(
                        out=wt5[:, c0 // 128 : c1 // 128, :],
                        in_=wb5_d[u, :, c0 // 128 : c1 // 128, :])
                wt16 = w16_pool.tile([128, B16COLS], bf16, tag="w16")
                nc.gpsimd.dma_start(out=wt16[:], in_=wb16_d[u])
                bias_t = bias_pool.tile([128, _BIAS_COLS], f32, tag="bias")
                nc.sync.dma_start(out=bias_t[:], in_=bias_d[u])

                # ---- upsample -> xupq (e5 hi/lo planes)
                xupq = xupq_pool.tile([128, 2 * NCH_XUP, BQ], e5, tag="xupq")
                for mc, (pairs, odd, base) in enumerate(_UP5[u % NP]):
                    ps = ps_pool.tile([128, BQ], f32, tag="ps")
                    _emit_fp8_group(nc, ps, wt5, xs_t, NCH_XS, pairs, odd, base)
                    rt = ract_pool.tile([128, BQ], bf16, tag="ract")
                    quant_passes(ps[:, :], None, rt, xupq, NCH_XUP, mc,
                                 relu=False)

                # ---- conv1 -> h1q (e5 hi/lo planes)
                h1q = h1q_pool.tile([128, 2 * NCH_H1, BQ], e5, tag="h1q")
                for mc, (pairs, odd, base) in enumerate(_SCHED5["c1"]):
                    ps = ps_pool.tile([128, BQ], f32, tag="ps")
                    _emit_fp8_group(nc, ps, wt5, xupq, NCH_XUP, pairs, odd,
                                    _C1_OFF + base)
                    rt = ract_pool.tile([128, BQ], bf16, tag="ract")
                    quant_passes(ps[:, :], bias_t[:, _BC1 + mc : _BC1 + mc + 1],
                                 rt, h1q, NCH_H1, mc, relu=True)

                # ---- conv2 (fp8) + interleaved conv3 (bf16) accumulation
                ps3 = ps3_pool.tile([128, BQ], f32, tag="ps3")

                def c3_mm(mc, h2t):
                    nc.tensor.matmul(
                        ps3[:mw3, :],
                        wt16[: kh3[mc], mc * 128 : mc * 128 + mw3],
                        h2t[: kh3[mc], :],
                        start=(mc == 0),
                        stop=(mc == n_c2 - 1),
                        skip_group_check=True,
                    )

                pending = None
                for mc, (pairs, odd, base) in enumerate(_SCHED5["c2"]):
                    ps = ps_pool.tile([128, BQ], f32, tag="ps")
                    _emit_fp8_group(nc, ps, wt5, h1q, NCH_H1, pairs, odd,
                                    _C1_OFF + base)
                    if pending is not None:
                        c3_mm(*pending)
                    h2t = h2_pool.tile([128, BQ], bf16, tag="h2")
                    beng = nc.scalar if mc % 2 == 0 else nc.vector
                    if hasattr(beng, "activation"):
                        beng.activation(h2t[:, :], ps[:, :], AF.Relu,
                                        bias=bias_t[:, _BC2 + mc : _BC2 + mc + 1])
                    else:
                        beng.tensor_scalar(h2t[:, :], ps[:, :],
                                           bias_t[:, _BC2 + mc : _BC2 + mc + 1],
                                           0.0, ALU.add, ALU.max)
                    pending = (mc, h2t)
                c3_mm(*pending)

                # ---- conv3 bias + store
                ot = out_pool.tile([128, BQ], f32, tag="out")
                nc.scalar.activation(
                    ot[:mw3, :], ps3[:mw3, :], AF.Identity,
                    bias=bias_t[:mw3, _BC3 : _BC3 + 1],
                )
                nc.sync.dma_start(out=y_d[u], in_=ot[:OUT_ROWS, :])

    nc.compile()
    return nc


# ----------------------------------------------------------- host pipeline
LAST_RESULTS = None


def _unit_weight_flats(inputs):
    w1 = np.stack([np.asarray(inputs["phi_w1"]), np.asarray(inputs["p_w1"])], 0)
    w2 = np.stack([np.asarray(inputs["phi_w2"]), np.asarray(inputs["p_w2"])], 0)
    w3 = np.stack([np.asarray(inputs["phi_w3"]), np.asarray(inputs["p_w3"])], 0)
    b1 = np.stack([np.asarray(inputs["phi_b1"]), np.asarray(inputs["p_b1"])], 0)
    b2 = np.stack([np.asarray(inputs["phi_b2"]), np.asarray(inputs["p_b2"])], 0)
    b3 = np.stack([np.asarray(inputs["phi_b3"]), np.asarray(inputs["p_b3"])], 0)
    w1f = w1.reshape(N_UNITS, -1).astype(np.float32)
    w2f = w2.reshape(N_UNITS, -1).astype(np.float32)
    w3f = w3.reshape(N_UNITS, -1).astype(np.float32)
    b1u = b1.reshape(N_UNITS, W1C).astype(np.float32)
    b2u = b2.reshape(N_UNITS, W2C).astype(np.float32)
    b3u = b3.reshape(N_UNITS).astype(np.float32)
    p = np.tile(np.arange(NP), 2)
    ij = np.stack([p // 5, p % 5], 1)
    return w1f, w2f, w3f, b1u, b2u, b3u, ij


def _build_host_inputs(inputs):
    x = np.asarray(inputs["x"], dtype=np.float32)
    w1f, w2f, w3f, b1u, b2u, b3u, ij = _unit_weight_flats(inputs)
    blocks = _build_f32_blocks(w1f, w2f, w3f, ij)
    w5, w16 = _build_wq(blocks, N_UNITS)

    ball = np.zeros((N_UNITS, 128, _BIAS_COLS), np.float32)
    for mc, (mw, _k) in enumerate(_SCHED["c1"]):
        ml = mc * 128 + np.arange(mw)
        ball[:, :mw, _BC1 + mc] = b1u[:, ml % W1C]
    for mc, (mw, _k) in enumerate(_SCHED["c2"]):
        ml = mc * 128 + np.arange(mw)
        ball[:, :mw, _BC2 + mc] = b2u[:, ml % W2C]
    ball[:, :OUT_ROWS, _BC3] = b3u[:, None]

    # xs slices per patch row i, quantized hi/lo e5, SBUF tile layout
    # [5, 128, 2*NCH_XS, B]: partition p, idx plane*NCH_XS + c <-> row c*128+p
    xt = np.ascontiguousarray(x.transpose(2, 3, 1, 0))  # [h, w, ci, b]
    xs_all = np.zeros((5, XS_PAD, B), np.float32)
    for i in range(5):
        h0 = _h0(i)
        xs_all[i, :XS_ROWS] = xt[h0 : h0 + H_WIN].reshape(XS_ROWS, B)
    xs_hi = xs_all.astype(E5)
    xs_lo = (xs_all - xs_hi.astype(np.float32)).astype(E5)
    # [5, XS_PAD, B] -> [5, NCH_XS, 128, B] -> [5, 128, NCH_XS, B]
    xs_q = np.zeros((5, 128, 2 * NCH_XS, B), dtype=E5)
    xs_q[:, :, :NCH_XS] = xs_hi.reshape(5, NCH_XS, 128, B).transpose(0, 2, 1, 3)
    xs_q[:, :, NCH_XS:] = xs_lo.reshape(5, NCH_XS, 128, B).transpose(0, 2, 1, 3)

    in_maps = []
    for c in range(N_CORES):
        br, q = c // 4, c % 4
        in_maps.append({
            "xs": np.ascontiguousarray(xs_q[:, :, :, q * BQ : (q + 1) * BQ]),
            "wb5": w5[br * NP : (br + 1) * NP],
            "wb16": w16[br * NP : (br + 1) * NP],
            "bias": ball[br * NP : (br + 1) * NP],
        })
    return in_maps


def _assemble(results):
    out = np.zeros((B, 2, OUT, OUT), np.float32)
    for c in range(N_CORES):
        br, q = c // 4, c % 4
        y = results[c]["y"]  # [25, 100, BQ]
        for p in range(NP):
            i, j = p // 5, p % 5
            blk = y[p].reshape(L, L, BQ).transpose(2, 0, 1)
            out[q * BQ : (q + 1) * BQ, br,
                10 * i : 10 * i + L, 10 * j : 10 * j + L] = blk
    return out


def kernel(**inputs):
    global _NC_CACHE, LAST_RESULTS
    in_maps = _build_host_inputs(inputs)
    if _NC_CACHE is None:
        _NC_CACHE = _build_nc()
    res = run_bass_kernel_spmd(_NC_CACHE, in_maps, list(range(N_CORES)))
    LAST_RESULTS = res
    return _assemble(res.results)


# ------------------------------------------------- numpy emulation (debug)
def q_bf16(a):
    return a.astype(BF16).astype(np.float32)


def emulate(**inputs):
    """Pure-numpy emulation of the device dataflow (v3 fp8 scheme)."""
    in_maps = _build_host_inputs(inputs)
    results = []
    for c in range(N_CORES):
        m = in_maps[c]
        w5 = m["wb5"]
        w16f = np.asarray(m["wb16"], dtype=np.float32)
        xsq = m["xs"]
        y = np.zeros((NP, OUT_ROWS, BQ), np.float32)
        for u in range(NP):
            # xs hi/lo planes [128, 2*NCH, BQ] -> [XS_PAD, BQ]
            def planes(qarr, nch):
                hi = qarr[:, :nch].transpose(1, 0, 2).reshape(nch * 128, BQ)
                lo = qarr[:, nch:].transpose(1, 0, 2).reshape(nch * 128, BQ)
                return hi.astype(np.float32), lo.astype(np.float32)

            xs_hi, xs_lo = planes(xsq[u // 5], NCH_XS)

            def fp8_layer(sched5, nch_in, hi, lo, col_shift, rows_out, nch_out):
                dst = np.zeros((rows_out, BQ), np.float32)
                w5u = np.asarray(w5[u]).reshape(128, Q5COLS)
                for mc, (pairs, odd, base) in enumerate(sched5):
                    col = col_shift + base
                    pacc = np.zeros((128, BQ), np.float32)

                    def blkf(c0):
                        return w5u[:, c0 : c0 + 128].astype(np.float32)

                    for (a, b) in pairs:
                        xha = hi[a * 128 : (a + 1) * 128]
                        xhb = hi[b * 128 : (b + 1) * 128]
                        xla = lo[a * 128 : (a + 1) * 128]
                        xlb = lo[b * 128 : (b + 1) * 128]
                        pacc += blkf(col).T @ xha + blkf(col + 128).T @ xhb
                        pacc += blkf(col + 256).T @ xha + blkf(col + 384).T @ xhb
                        pacc += blkf(col).T @ xla + blkf(col + 128).T @ xlb
                        col += 512
                    if odd is not None:
                        xho = hi[odd * 128 : (odd + 1) * 128]
                        xlo_ = lo[odd * 128 : (odd + 1) * 128]
                        pacc += blkf(col).T @ xho + blkf(col + 128).T @ xlo_
                        pacc += blkf(col + 256).T @ xho
                    r0 = mc * 128
                    dst[r0 : min(r0 + 128, rows_out)] = pacc[: min(128, rows_out - r0)]
                return dst

            xup = fp8_layer(_UP5[u % NP], NCH_XS, xs_hi, xs_lo, 0,
                            XUP_ROWS, NCH_XUP)
            # quantize xup (pad to chunks)
            xup_p = np.zeros((NCH_XUP * 128, BQ), np.float32)
            xup_p[:XUP_ROWS] = xup
            r = q_bf16(xup_p)
            hi = r.astype(E5).astype(np.float32)
            lo = (r - hi).astype(E5).astype(np.float32)

            h1 = fp8_layer(_SCHED5["c1"], NCH_XUP, hi, lo, _C1_OFF,
                           NCH_H1 * 128, NCH_H1)
            b1 = m["bias"][u][np.arange(H1_ROWS) % 128,
                              _BC1 + np.arange(H1_ROWS) // 128]
            h1[:H1_ROWS] = np.maximum(h1[:H1_ROWS] + b1[:, None], 0)
            h1[H1_ROWS:] = 0
            r = q_bf16(h1)
            hi = r.astype(E5).astype(np.float32)
            lo = (r - hi).astype(E5).astype(np.float32)

            h2 = fp8_layer(_SCHED5["c2"], NCH_H1, hi, lo, _C1_OFF,
                           H2_ROWS, NCH_H2)
            b2 = m["bias"][u][np.arange(H2_ROWS) % 128,
                              _BC2 + np.arange(H2_ROWS) // 128]
            h2 = q_bf16(np.maximum(h2 + b2[:, None], 0))

            out3 = np.zeros((OUT_ROWS, BQ), np.float32)
            for idx, (kc, kh) in enumerate(_SCHED["c3"][0][1]):
                out3 += w16f[u][:kh, idx * 128 : idx * 128 + OUT_ROWS].T @ \
                    h2[kc * 128 : kc * 128 + kh]
            out3 += m["bias"][u][:OUT_ROWS, _BC3][:, None]
            y[u] = out3
        results.append({"y": y})
    return _assemble(results)
